# revision 25
# baseline (speedup 1.0000x reference)
"""
Trainium2 Bass kernel for nn_CVXPolicy_DoubleIntegrator (131072 x 192 -> 131072 x 96).

Math:
  p  = MLP(concat([t, z]))     # tanh x2, hidden 100
  q  = p @ S                   # scatter fold, S is 0/1 [192, 96]
  u* = -q / (1 + s),  s(1+s)^2 = ||q||^2 per row (strictly convex cubic)

Strategy (8 cores, data parallel over the batch; all layout shuffles on host):
  - Host: per-core shard -> transposed, tile-major superblocks
    zTt [NSUP, 193, SUPN]: rows 0:192 = z features, row 192 = t. One DMA
    per superblock per operand group; output written as uT [NSUP, 96, SUPN].
  - Device: transposed space (features on partitions, rows on free dim),
    32 blocks of N=512 (PSUM bank width):
      h1 = tanh(W1'^T x + b1)        (ACT per-partition bias, fused)
      h2 = tanh(W2^T h1 + b2)
      q- = W3'^T h2      where W3' = -(W3 @ S)  (negation folded)
      q  = q- + b3'  via ACT Identity+bias while copying PSUM->SBUF
      r  = sum_f q^2 accumulated across the group's blocks into one PSUM
           tile [16, 512] via one-hot selector matmuls (row j = block j)
      s  : closed-form Cardano (cbrt via exp(ln/3)) + Newton polish
      c  = 1/(1+s) split into f32r hi+lo, broadcast back to [96, 512] via a
           single K-stacked selector matmul; uT = q * c on DVE.
  - Matmuls run in dtype float32r (full-rate fp32-reduced on the PE at
    N>=256); operands are written as f32r by their producer engines. The
    c broadcast uses an exact hi+lo two-term representation.

Self-contained: hardcodes shapes; only needs numpy + the concourse tree.
"""

import os
import sys
import numpy as np
from contextlib import ExitStack

for _p in ("/opt/trn_rl_repo", "/root/.axon_site/_ro/trn_rl_repo"):
    if _p not in sys.path and os.path.isdir(_p):
        sys.path.append(_p)

B_TOTAL = 131072
N_CORES = 8
BC = B_TOTAL // N_CORES          # 16384 rows per core
SD = 192
CD = 96
HID = 100
BLKN = 512                       # rows per block (PSUM bank width in fp32)
NBLK = BC // BLKN                # 32
SUPER = int(os.environ.get("K_SUPER", "4"))   # blocks per DMA superblock
NSUP = NBLK // SUPER
SUPN = SUPER * BLKN
NG = int(os.environ.get("K_GROUPS", "2"))     # groups for pipelined tail
GBLK = NBLK // NG                # blocks per group
NEWTON_ITERS = int(os.environ.get("K_NEWTON", "1"))
USE_F32R = os.environ.get("K_F32R", "1") == "1"
REPEAT = int(os.environ.get("K_REPEAT", "1"))

_PROG_CACHE = {}


def _scatter_matrix():
    n = 32
    u_idx = np.arange(n)
    p_idx = np.concatenate([3 * np.arange(1, n + 1), 4 * np.arange(1, n + 1),
                            5 * np.arange(1, n + 1)])
    uu_idx = np.concatenate([u_idx, 2 * u_idx, 3 * u_idx])
    S = np.zeros((SD, CD), np.float64)
    for pi, ui in zip(p_idx, uu_idx):
        S[pi, ui] += 1.0
    return S


def _build_program():
    import concourse.mybir as mybir
    from concourse import bacc
    from concourse.tile import TileContext

    f32 = mybir.dt.float32
    f32r = mybir.dt.float32r
    FT = mybir.ActivationFunctionType
    ALU = mybir.AluOpType
    fnp = np.float32

    mmdt = f32r if USE_F32R else f32

    nc = bacc.Bacc("TRN2", target_bir_lowering=False, debug=False,
                   num_devices=N_CORES)

    def din(name, shape, dt=f32):
        return nc.dram_tensor(name, shape, dt, kind="ExternalInput").ap()

    zTt_d = din("zTt", [NSUP, SD + 1, SUPN], mmdt)
    w1a_d = din("w1a", [128, HID], mmdt)
    w1b_d = din("w1b", [65, HID], mmdt)
    w2_d = din("w2", [HID, HID], mmdt)
    w3_d = din("w3", [HID, CD], mmdt)
    b1_d = din("b1", [HID, 1])
    b2_d = din("b2", [HID, 1])
    b3_d = din("b3", [CD, 1])
    rsel_d = din("rsel", [CD, GBLK * GBLK], mmdt)
    sels_d = din("sels", [32 + GBLK, GBLK * CD], mmdt)
    uT_d = nc.dram_tensor("uT", [NSUP, CD, SUPN], f32,
                          kind="ExternalOutput").ap()

    with TileContext(nc) as tc, ExitStack() as ctx:
        ctx.enter_context(nc.allow_low_precision(
            reason="float32r matmul operands; accumulation stays fp32 PSUM"))
        consts = ctx.enter_context(tc.tile_pool(name="consts", bufs=1))
        w1a = consts.tile([128, HID], mmdt)
        w1b = consts.tile([65, HID], mmdt)
        w2 = consts.tile([HID, HID], mmdt)
        w3 = consts.tile([HID, CD], mmdt)
        b1 = consts.tile([HID, 1], f32)
        b2 = consts.tile([HID, 1], f32)
        b3c = consts.tile([CD, 1], f32)
        rsel = consts.tile([CD, GBLK * GBLK], mmdt)
        sels = consts.tile([32 + GBLK, GBLK * CD], mmdt)
        for sb, dr in [(w1a, w1a_d), (w1b, w1b_d), (w2, w2_d), (w3, w3_d),
                       (b1, b1_d), (b2, b2_d), (b3c, b3_d), (rsel, rsel_d),
                       (sels, sels_d)]:
            nc.sync.dma_start(out=sb[:], in_=dr[:])

        # persistent q for the whole core: 96 x 16384 f32 = 6.3 MB
        qsb = consts.tile([CD, BC], f32)

        xpool = ctx.enter_context(tc.tile_pool(name="x", bufs=3))
        hpool = ctx.enter_context(tc.tile_pool(name="h", bufs=4))
        opool = ctx.enter_context(tc.tile_pool(name="o", bufs=3))
        ppool = ctx.enter_context(tc.tile_pool(name="p2", bufs=2))
        mmps = ctx.enter_context(tc.tile_pool(name="mmps", bufs=2, space="PSUM"))
        rps_pool = ctx.enter_context(tc.tile_pool(name="rps", bufs=1,
                                                  space="PSUM"))
        cps_pool = ctx.enter_context(tc.tile_pool(name="cps", bufs=2,
                                                  space="PSUM"))

        def phase2(rps):
            """rps: PSUM [GBLK, BLKN] holding r.
            Returns SBUF [2*GBLK, BLKN] f32r: rows 0:G = c_hi, G:2G = c_lo,
            where c = 1/(1+s)."""
            def tmp():
                return ppool.tile([GBLK, BLKN], f32, tag="p2t", bufs=8,
                                  name="p2tmp")
            r = ppool.tile([GBLK, BLKN], f32, tag="p2r", bufs=2, name="p2r")
            nc.vector.tensor_copy(out=r[:], in_=rps[:])
            qp = tmp()
            nc.vector.tensor_scalar_add(qp[:], r[:], float(fnp(2.0 / 27.0)))
            sq = tmp()
            nc.vector.tensor_tensor(out=sq[:], in0=qp[:], in1=qp[:], op=ALU.mult)
            d = tmp()
            nc.vector.tensor_scalar(out=d[:], in0=sq[:],
                                    scalar1=float(fnp(-4.0 / 729.0)),
                                    scalar2=0.0, op0=ALU.add, op1=ALU.max)
            # sqrt(d) = exp(0.5 ln d); d=0 -> ln -inf -> exp -> 0
            lnd = tmp()
            nc.scalar.activation(out=lnd[:], in_=d[:], func=FT.Ln)
            sd = tmp()
            nc.scalar.activation(out=sd[:], in_=lnd[:], func=FT.Exp, scale=0.5)
            u32 = tmp()
            nc.vector.tensor_tensor(out=u32[:], in0=qp[:], in1=sd[:], op=ALU.add)
            lnu = tmp()
            nc.scalar.activation(out=lnu[:], in_=u32[:], func=FT.Ln, scale=0.5)
            u = tmp()
            nc.scalar.activation(out=u[:], in_=lnu[:], func=FT.Exp,
                                 scale=float(fnp(1.0 / 3.0)))
            ru = tmp()
            nc.vector.reciprocal(out=ru[:], in_=u[:])
            sv = tmp()
            nc.vector.tensor_scalar(out=sv[:], in0=ru[:],
                                    scalar1=float(fnp(1.0 / 9.0)),
                                    scalar2=float(fnp(-2.0 / 3.0)),
                                    op0=ALU.mult, op1=ALU.add)
            s = ppool.tile([GBLK, BLKN], f32, tag="p2s", bufs=2, name="p2s")
            nc.vector.tensor_tensor(out=s[:], in0=sv[:], in1=u[:], op=ALU.add)
            nc.vector.tensor_scalar_max(s[:], s[:], 0.0)
            for _ in range(NEWTON_ITERS):
                one = tmp()
                nc.vector.tensor_scalar_add(one[:], s[:], 1.0)
                o2 = tmp()
                nc.vector.tensor_tensor(out=o2[:], in0=one[:], in1=one[:],
                                        op=ALU.mult)
                num = tmp()
                nc.vector.tensor_tensor(out=num[:], in0=s[:], in1=o2[:],
                                        op=ALU.mult)
                nc.vector.tensor_tensor(out=num[:], in0=num[:], in1=r[:],
                                        op=ALU.subtract)
                t3 = tmp()
                nc.vector.tensor_scalar(out=t3[:], in0=s[:], scalar1=3.0,
                                        scalar2=1.0, op0=ALU.mult, op1=ALU.add)
                den = tmp()
                nc.vector.tensor_tensor(out=den[:], in0=one[:], in1=t3[:],
                                        op=ALU.mult)
                rden = tmp()
                nc.vector.reciprocal(out=rden[:], in_=den[:])
                quo = tmp()
                nc.vector.tensor_tensor(out=quo[:], in0=num[:], in1=rden[:],
                                        op=ALU.mult)
                nc.vector.tensor_tensor(out=s[:], in0=s[:], in1=quo[:],
                                        op=ALU.subtract)
                nc.vector.tensor_scalar_max(s[:], s[:], 0.0)
            onep = tmp()
            nc.vector.tensor_scalar_add(onep[:], s[:], 1.0)
            cf = tmp()
            nc.vector.reciprocal(out=cf[:], in_=onep[:])
            # exact two-term f32r representation: c = hi + lo.
            # Engine writes must start at 32-aligned partitions, so the lo
            # half sits at partition 32 (rows 16:32 stay zero, sels matches).
            cstack = ppool.tile([32 + GBLK, BLKN], mmdt, tag="p2c", bufs=2,
                                name="cstack")
            nc.vector.memset(cstack[:].bitcast(f32), 0.0)
            nc.vector.tensor_copy(out=cstack[0:GBLK, :], in_=cf[:])
            nc.vector.tensor_tensor(out=cstack[32:32 + GBLK, :], in0=cf[:],
                                    in1=cstack[0:GBLK, :], op=ALU.subtract)
            return cstack

        # ---- software-pipelined emission ----
        # Per-block stage chain: L1(PE) -> tanh1(ACT) -> L2(PE) -> tanh2(ACT)
        # -> L3(PE) -> q+bias(ACT) -> qsq(GPSIMD) -> r(PE). Emitting stage s
        # of block T-s at tick T gives every cross-engine edge a one-tick
        # slack, so the in-order engine queues stream without head-of-line
        # stalls. Phase 3 of group g-1 is interleaved into group g's ticks
        # (offset OFF3) so its cb-matmuls sit in the PE queue only after
        # phase 2 of g-1 has had time to finish on DVE/ACT.
        STAGES = 8
        OFF3 = 9

        st_h1p, st_h1, st_h2p, st_h2, st_qp, st_qsq = {}, {}, {}, {}, {}, {}
        st_xa, st_xb = {}, {}
        st_cb, st_ut = {}, {}

        def ph1_tick(g, T, rps):
            # stage 0: loads + L1
            j = T
            if 0 <= j < GBLK:
                b = g * GBLK + j
                su, so = divmod(b, SUPER)
                if so == 0:
                    xa = xpool.tile([128, SUPN], mmdt, tag="xa", name="xa")
                    nc.sync.dma_start(out=xa[:], in_=zTt_d[su, 0:128, :])
                    xb = xpool.tile([65, SUPN], mmdt, tag="xb", name="xb")
                    nc.sync.dma_start(out=xb[:], in_=zTt_d[su, 128:193, :])
                    st_xa[su], st_xb[su] = xa, xb
                ns = slice(so * BLKN, (so + 1) * BLKN)
                h1p = mmps.tile([HID, BLKN], f32, tag="h1p", bufs=2, name="h1p")
                nc.tensor.matmul(h1p[:], w1a[:], st_xa[su][:, ns],
                                 start=True, stop=False)
                nc.tensor.matmul(h1p[:], w1b[:], st_xb[su][:, ns],
                                 start=False, stop=True)
                st_h1p[j] = h1p
            # stage 1: tanh1
            j = T - 1
            if 0 <= j < GBLK:
                h1 = hpool.tile([HID, BLKN], mmdt, tag="h1", bufs=3, name="h1")
                nc.scalar.activation(out=h1[:], in_=st_h1p.pop(j)[:],
                                     func=FT.Tanh, bias=b1[:])
                st_h1[j] = h1
            # stage 2: L2
            j = T - 2
            if 0 <= j < GBLK:
                h2p = mmps.tile([HID, BLKN], f32, tag="h2p", bufs=2, name="h2p")
                nc.tensor.matmul(h2p[:], w2[:], st_h1.pop(j)[:],
                                 start=True, stop=True)
                st_h2p[j] = h2p
            # stage 3: tanh2
            j = T - 3
            if 0 <= j < GBLK:
                h2 = hpool.tile([HID, BLKN], mmdt, tag="h2", bufs=3, name="h2")
                nc.scalar.activation(out=h2[:], in_=st_h2p.pop(j)[:],
                                     func=FT.Tanh, bias=b2[:])
                st_h2[j] = h2
            # stage 4: L3
            j = T - 4
            if 0 <= j < GBLK:
                qp = mmps.tile([CD, BLKN], f32, tag="qp", bufs=1, name="qp")
                nc.tensor.matmul(qp[:], w3[:], st_h2.pop(j)[:],
                                 start=True, stop=True)
                st_qp[j] = qp
            # stage 5: q = q- + b3 (PSUM -> persistent SBUF), on DVE
            j = T - 5
            if 0 <= j < GBLK:
                n0 = (g * GBLK + j) * BLKN
                nc.vector.tensor_scalar(out=qsb[:, n0:n0 + BLKN],
                                        in0=st_qp.pop(j)[:], scalar1=b3c[:],
                                        scalar2=None, op0=ALU.add)
            # stage 6: qsq on gpsimd
            j = T - 6
            if 0 <= j < GBLK:
                n0 = (g * GBLK + j) * BLKN
                qsq = hpool.tile([CD, BLKN], mmdt, tag="qsq", bufs=3,
                                 name="qsq")
                nc.gpsimd.tensor_tensor(out=qsq[:], in0=qsb[:, n0:n0 + BLKN],
                                        in1=qsb[:, n0:n0 + BLKN], op=ALU.mult)
                st_qsq[j] = qsq
            # stage 7: r accumulation
            j = T - 7
            if 0 <= j < GBLK:
                nc.tensor.matmul(rps[:], rsel[:, GBLK * j:GBLK * (j + 1)],
                                 st_qsq.pop(j)[:], start=(j == 0),
                                 stop=(j == GBLK - 1))

        def ph3_tick(g, T, cstack):
            # stage A: cb broadcast matmul
            j = T
            if 0 <= j < GBLK:
                b = g * GBLK + j
                su, so = divmod(b, SUPER)
                if so == 0:
                    st_ut[su] = opool.tile([CD, SUPN], f32, tag="ut", bufs=3,
                                           name="ut")
                cb = cps_pool.tile([CD, BLKN], f32, tag="cb", name="cb")
                nc.tensor.matmul(cb[:], sels[:, CD * j:CD * (j + 1)],
                                 cstack[:], start=True, stop=True)
                st_cb[j] = cb
            # stage B: ut multiply + store
            j = T - 1
            if 0 <= j < GBLK:
                b = g * GBLK + j
                n0 = b * BLKN
                su, so = divmod(b, SUPER)
                ns = slice(so * BLKN, (so + 1) * BLKN)
                nc.vector.tensor_tensor(out=st_ut[su][:, ns],
                                        in0=qsb[:, n0:n0 + BLKN],
                                        in1=st_cb.pop(j)[:], op=ALU.mult)
                if so == SUPER - 1 or j == GBLK - 1:
                    nc.sync.dma_start(out=uT_d[su], in_=st_ut.pop(su)[:])

        for _rep in range(REPEAT):
            prev = None                      # (g-1, cstack) awaiting phase 3
            for g in range(NG):
                rps = rps_pool.tile([GBLK, BLKN], f32, tag="rps", name="rps")
                for T in range(GBLK + STAGES):
                    ph1_tick(g, T, rps)
                    if prev is not None:
                        ph3_tick(prev[0], T - OFF3, prev[1])
                cstack = phase2(rps)
                if prev is not None:
                    for T in range(GBLK + STAGES - OFF3, GBLK + 1):
                        ph3_tick(prev[0], T, prev[1])
                prev = (g, cstack)
            # drain last group's phase 3
            for T in range(GBLK + 1):
                ph3_tick(prev[0], T, prev[1])

    nc.compile()
    return nc


def _host_constants(W1, b1, W2, b2, W3, b3):
    S = _scatter_matrix()
    W1z = W1[1:, :].astype(np.float32)          # [192, 100]
    W1t = W1[0:1, :].astype(np.float32)         # [1, 100]
    w1a = np.ascontiguousarray(W1z[0:128])
    w1b = np.ascontiguousarray(np.concatenate([W1z[128:192], W1t], axis=0))
    w3 = np.ascontiguousarray(-(W3.astype(np.float64) @ S)).astype(np.float32)
    b3n = (-(b3.astype(np.float64) @ S)).astype(np.float32).reshape(CD, 1)
    rsel = np.zeros((CD, GBLK * GBLK), np.float32)
    for j in range(GBLK):
        rsel[:, GBLK * j + j] = 1.0
    sels = np.zeros((32 + GBLK, GBLK * CD), np.float32)
    for j in range(GBLK):
        sels[j, CD * j:CD * (j + 1)] = 1.0            # hi part
        sels[32 + j, CD * j:CD * (j + 1)] = 1.0       # lo part (aligned @32)
    return {
        "w1a": w1a,
        "w1b": w1b,
        "w2": np.ascontiguousarray(W2.astype(np.float32)),
        "w3": w3,
        "b1": np.ascontiguousarray(b1.astype(np.float32).reshape(HID, 1)),
        "b2": np.ascontiguousarray(b2.astype(np.float32).reshape(HID, 1)),
        "b3": np.ascontiguousarray(b3n),
        "rsel": rsel,
        "sels": sels,
    }


def _shard_inputs(z, t, consts):
    in_maps = []
    for c in range(N_CORES):
        sl = slice(c * BC, (c + 1) * BC)
        m = dict(consts)
        xt = np.concatenate([z[sl].T, t[sl].reshape(1, BC)], axis=0)  # [193,BC]
        xt = xt.reshape(SD + 1, NSUP, SUPN).transpose(1, 0, 2)
        m["zTt"] = np.ascontiguousarray(xt)       # [NSUP, 193, SUPN]
        in_maps.append(m)
    return in_maps


def _unshard_output(uT):
    """uT [NSUP, CD, SUPN] -> [BC, CD]."""
    full = np.asarray(uT).transpose(1, 0, 2).reshape(CD, BC)
    return np.ascontiguousarray(full.T)


def _get_program():
    key = (USE_F32R, NEWTON_ITERS, NG, REPEAT, SUPER)
    if key not in _PROG_CACHE:
        _PROG_CACHE[key] = _build_program()
    return _PROG_CACHE[key]


def kernel(z, t, W1, b1, W2, b2, W3, b3, _trace=False):
    from concourse.bass_utils import run_bass_kernel_spmd

    z = np.asarray(z, np.float32)
    t = np.asarray(t, np.float32)
    consts = _host_constants(np.asarray(W1), np.asarray(b1), np.asarray(W2),
                             np.asarray(b2), np.asarray(W3), np.asarray(b3))
    nc = _get_program()
    in_maps = _shard_inputs(z, t, consts)
    res = run_bass_kernel_spmd(nc, in_maps, list(range(N_CORES)), trace=_trace)
    outs = [_unshard_output(res.results[c]["uT"]) for c in range(N_CORES)]
    u = np.concatenate(outs, axis=0).astype(np.float32)
    if _trace:
        return u, res
    return u


def _make_runner(in_maps):
    """Build the sharded PJRT callable (same lowering as run_bass_via_pjrt)
    and put inputs on device once, for steady-state timing."""
    import jax
    import numpy as _np
    from jax.sharding import Mesh, PartitionSpec
    from jax.experimental.shard_map import shard_map
    import concourse.mybir as mybir
    from concourse import bass2jax

    nc = _get_program()
    bass2jax.install_neuronx_cc_hook()

    partition_name = (nc.partition_id_tensor.name
                      if nc.partition_id_tensor else None)
    in_names, out_names, out_avals, zero_outs = [], [], [], []
    for alloc in nc.m.functions[0].allocations:
        if not isinstance(alloc, mybir.MemoryLocationSet):
            continue
        name = alloc.memorylocations[0].name
        if alloc.kind == "ExternalInput":
            if name != partition_name:
                in_names.append(name)
        elif alloc.kind == "ExternalOutput":
            shape = list(alloc.tensor_shape)
            dt = mybir.dt.np(alloc.dtype)
            out_names.append(name)
            out_avals.append(jax.core.ShapedArray(shape, dt))
            zero_outs.append(_np.zeros(shape, dt))
    in_names_full = in_names + out_names
    if partition_name is not None:
        in_names_full.append(partition_name)

    def _body(*args):
        operands = list(args)
        if partition_name is not None:
            operands.append(bass2jax.partition_id_tensor())
        outs = bass2jax._bass_exec_p.bind(
            *operands,
            out_avals=tuple(out_avals),
            in_names=tuple(in_names_full),
            out_names=tuple(out_names),
            lowering_input_output_aliases=(),
            sim_require_finite=True,
            sim_require_nnan=True,
            nc=nc,
        )
        return tuple(outs)

    devices = jax.devices()[:N_CORES]
    mesh = Mesh(np.asarray(devices), ("core",))
    nin = len(in_names) + len(zero_outs)
    fn = jax.jit(shard_map(_body, mesh=mesh,
                           in_specs=(PartitionSpec("core"),) * nin,
                           out_specs=(PartitionSpec("core"),) * len(out_names),
                           check_rep=False), keep_unused=True)
    concat = [_np.concatenate([in_maps[c][n] for c in range(N_CORES)], axis=0)
              for n in in_names]
    concat += [_np.zeros((N_CORES * z.shape[0], *z.shape[1:]), z.dtype)
               for z in zero_outs]
    sh = jax.sharding.NamedSharding(mesh, PartitionSpec("core"))
    dev_in = [jax.device_put(a, sh) for a in concat]
    return fn, dev_in, out_names


def bench(z, t, W1, b1, W2, b2, W3, b3, iters=20):
    """Returns (per-iteration wall ns, outputs dict) at steady state."""
    import time as _time
    import jax
    z = np.asarray(z, np.float32)
    t = np.asarray(t, np.float32)
    consts = _host_constants(np.asarray(W1), np.asarray(b1), np.asarray(W2),
                             np.asarray(b2), np.asarray(W3), np.asarray(b3))
    in_maps = _shard_inputs(z, t, consts)
    fn, dev_in, out_names = _make_runner(in_maps)
    out = fn(*dev_in)
    jax.block_until_ready(out)
    t0 = _time.perf_counter()
    for _ in range(iters):
        out = fn(*dev_in)
    jax.block_until_ready(out)
    t1 = _time.perf_counter()
    return (t1 - t0) / iters * 1e9, dict(zip(out_names, out))


# revision 26
# speedup vs baseline: 1.0395x; 1.0395x over previous
"""
Trainium2 Bass kernel for nn_CVXPolicy_DoubleIntegrator (131072 x 192 -> 131072 x 96).

Math:
  p  = MLP(concat([t, z]))     # tanh x2, hidden 100
  q  = p @ S                   # scatter fold, S is 0/1 [192, 96]
  u* = -q / (1 + s),  s(1+s)^2 = ||q||^2 per row (strictly convex cubic)

Strategy (8 cores, data parallel over the batch; all layout shuffles on host):
  - Host: per-core shard -> transposed, tile-major superblocks
    zTt [NSUP, 193, SUPN]: rows 0:192 = z features, row 192 = t. One DMA
    per superblock per operand group; output written as uT [NSUP, 96, SUPN].
  - Device: transposed space (features on partitions, rows on free dim),
    32 blocks of N=512 (PSUM bank width):
      h1 = tanh(W1'^T x + b1)        (ACT per-partition bias, fused)
      h2 = tanh(W2^T h1 + b2)
      q- = W3'^T h2      where W3' = -(W3 @ S)  (negation folded)
      q  = q- + b3'  via ACT Identity+bias while copying PSUM->SBUF
      r  = sum_f q^2 accumulated across the group's blocks into one PSUM
           tile [16, 512] via one-hot selector matmuls (row j = block j)
      s  : closed-form Cardano (cbrt via exp(ln/3)) + Newton polish
      c  = 1/(1+s) split into f32r hi+lo, broadcast back to [96, 512] via a
           single K-stacked selector matmul; uT = q * c on DVE.
  - Matmuls run in dtype float32r (full-rate fp32-reduced on the PE at
    N>=256); operands are written as f32r by their producer engines. The
    c broadcast uses an exact hi+lo two-term representation.

Self-contained: hardcodes shapes; only needs numpy + the concourse tree.
"""

import os
import sys
import numpy as np
from contextlib import ExitStack

for _p in ("/opt/trn_rl_repo", "/root/.axon_site/_ro/trn_rl_repo"):
    if _p not in sys.path and os.path.isdir(_p):
        sys.path.append(_p)

B_TOTAL = 131072
N_CORES = 8
BC = B_TOTAL // N_CORES          # 16384 rows per core
SD = 192
CD = 96
HID = 100
BLKN = 512                       # rows per block (PSUM bank width in fp32)
NBLK = BC // BLKN                # 32
SUPER = int(os.environ.get("K_SUPER", "4"))   # blocks per DMA superblock
NSUP = NBLK // SUPER
SUPN = SUPER * BLKN
NG = int(os.environ.get("K_GROUPS", "2"))     # groups for pipelined tail
GBLK = NBLK // NG                # blocks per group
NEWTON_ITERS = int(os.environ.get("K_NEWTON", "1"))
USE_F32R = os.environ.get("K_F32R", "1") == "1"
REPEAT = int(os.environ.get("K_REPEAT", "1"))

_PROG_CACHE = {}


def _scatter_matrix():
    n = 32
    u_idx = np.arange(n)
    p_idx = np.concatenate([3 * np.arange(1, n + 1), 4 * np.arange(1, n + 1),
                            5 * np.arange(1, n + 1)])
    uu_idx = np.concatenate([u_idx, 2 * u_idx, 3 * u_idx])
    S = np.zeros((SD, CD), np.float64)
    for pi, ui in zip(p_idx, uu_idx):
        S[pi, ui] += 1.0
    return S


def _build_program():
    import concourse.mybir as mybir
    from concourse import bacc
    from concourse.tile import TileContext

    f32 = mybir.dt.float32
    f32r = mybir.dt.float32r
    FT = mybir.ActivationFunctionType
    ALU = mybir.AluOpType
    fnp = np.float32

    mmdt = f32r if USE_F32R else f32

    nc = bacc.Bacc("TRN2", target_bir_lowering=False, debug=False,
                   num_devices=N_CORES)

    def din(name, shape, dt=f32):
        return nc.dram_tensor(name, shape, dt, kind="ExternalInput").ap()

    zTt_d = din("zTt", [NSUP, SD + 1, SUPN], mmdt)
    w1a_d = din("w1a", [128, HID], mmdt)
    w1b_d = din("w1b", [65, HID], mmdt)
    w2_d = din("w2", [HID, HID], mmdt)
    w3_d = din("w3", [HID, CD], mmdt)
    b1_d = din("b1", [HID, 1])
    b2_d = din("b2", [HID, 1])
    b3_d = din("b3", [CD, 1])
    rsel_d = din("rsel", [CD, GBLK * GBLK], mmdt)
    sels_d = din("sels", [32 + GBLK, GBLK * CD], mmdt)
    uT_d = nc.dram_tensor("uT", [NSUP, CD, SUPN], f32,
                          kind="ExternalOutput").ap()

    with TileContext(nc) as tc, ExitStack() as ctx:
        ctx.enter_context(nc.allow_low_precision(
            reason="float32r matmul operands; accumulation stays fp32 PSUM"))
        consts = ctx.enter_context(tc.tile_pool(name="consts", bufs=1))
        w1a = consts.tile([128, HID], mmdt)
        w1b = consts.tile([65, HID], mmdt)
        w2 = consts.tile([HID, HID], mmdt)
        w3 = consts.tile([HID, CD], mmdt)
        b1 = consts.tile([HID, 1], f32)
        b2 = consts.tile([HID, 1], f32)
        b3c = consts.tile([CD, 1], f32)
        rsel = consts.tile([CD, GBLK * GBLK], mmdt)
        sels = consts.tile([32 + GBLK, GBLK * CD], mmdt)
        for sb, dr in [(w1a, w1a_d), (w1b, w1b_d), (w2, w2_d), (w3, w3_d),
                       (b1, b1_d), (b2, b2_d), (b3c, b3_d), (rsel, rsel_d),
                       (sels, sels_d)]:
            nc.scalar.dma_start(out=sb[:], in_=dr[:])

        # persistent q for the whole core: 96 x 16384 f32 = 6.3 MB
        qsb = consts.tile([CD, BC], f32)

        xpool = ctx.enter_context(tc.tile_pool(name="x", bufs=3))
        hpool = ctx.enter_context(tc.tile_pool(name="h", bufs=4))
        opool = ctx.enter_context(tc.tile_pool(name="o", bufs=3))
        ppool = ctx.enter_context(tc.tile_pool(name="p2", bufs=2))
        mmps = ctx.enter_context(tc.tile_pool(name="mmps", bufs=2, space="PSUM"))
        rps_pool = ctx.enter_context(tc.tile_pool(name="rps", bufs=1,
                                                  space="PSUM"))
        cps_pool = ctx.enter_context(tc.tile_pool(name="cps", bufs=2,
                                                  space="PSUM"))

        def phase2(rps):
            """rps: PSUM [GBLK, BLKN] holding r. Two independent half-width
            chains (FD=256) interleave on DVE/ACT, halving serial latency.
            Returns SBUF [32+GBLK, BLKN] f32r: rows 0:G = c_hi, 32:32+G = c_lo,
            where c = 1/(1+s)."""
            H = BLKN // 2
            cstack = ppool.tile([32 + GBLK, BLKN], mmdt, tag="p2c", bufs=2,
                                name="cstack")
            nc.vector.memset(cstack[:].bitcast(f32), 0.0)
            r_t = ppool.tile([GBLK, BLKN], f32, tag="p2r", bufs=2, name="p2r")
            s_t = ppool.tile([GBLK, BLKN], f32, tag="p2s", bufs=2, name="p2s")
            halves = [slice(0, H), slice(H, BLKN)]

            def tmp():
                return ppool.tile([GBLK, H], f32, tag="p2t", bufs=16,
                                  name="p2tmp")

            st = [{} for _ in halves]
            for hi, hs in enumerate(halves):
                st[hi]["r"] = r_t[:, hs]
                st[hi]["s"] = s_t[:, hs]

            def emit(fn):
                for hi, hs in enumerate(halves):
                    fn(st[hi], hs)

            def c_copy(v, hs):
                nc.vector.tensor_copy(out=v["r"], in_=rps[:, hs])
            emit(c_copy)

            def c_qp(v, hs):
                v["qp"] = tmp()
                nc.vector.tensor_scalar_add(v["qp"][:], v["r"],
                                            float(fnp(2.0 / 27.0)))
            emit(c_qp)

            def c_sq(v, hs):
                v["sq"] = tmp()
                nc.vector.tensor_tensor(out=v["sq"][:], in0=v["qp"][:],
                                        in1=v["qp"][:], op=ALU.mult)
            emit(c_sq)

            def c_d(v, hs):
                v["d"] = tmp()
                nc.vector.tensor_scalar(out=v["d"][:], in0=v["sq"][:],
                                        scalar1=float(fnp(-4.0 / 729.0)),
                                        scalar2=0.0, op0=ALU.add, op1=ALU.max)
            emit(c_d)

            def c_lnd(v, hs):
                v["lnd"] = tmp()
                nc.scalar.activation(out=v["lnd"][:], in_=v["d"][:], func=FT.Ln)
            emit(c_lnd)

            def c_sd(v, hs):
                v["sd"] = tmp()
                nc.scalar.activation(out=v["sd"][:], in_=v["lnd"][:],
                                     func=FT.Exp, scale=0.5)
            emit(c_sd)

            def c_u32(v, hs):
                v["u32"] = tmp()
                nc.vector.tensor_tensor(out=v["u32"][:], in0=v["qp"][:],
                                        in1=v["sd"][:], op=ALU.add)
            emit(c_u32)

            def c_lnu(v, hs):
                v["lnu"] = tmp()
                nc.scalar.activation(out=v["lnu"][:], in_=v["u32"][:],
                                     func=FT.Ln, scale=0.5)
            emit(c_lnu)

            def c_u(v, hs):
                v["u"] = tmp()
                nc.scalar.activation(out=v["u"][:], in_=v["lnu"][:],
                                     func=FT.Exp, scale=float(fnp(1.0 / 3.0)))
            emit(c_u)

            def c_ru(v, hs):
                v["ru"] = tmp()
                nc.vector.reciprocal(out=v["ru"][:], in_=v["u"][:])
            emit(c_ru)

            def c_sv(v, hs):
                v["sv"] = tmp()
                nc.vector.tensor_scalar(out=v["sv"][:], in0=v["ru"][:],
                                        scalar1=float(fnp(1.0 / 9.0)),
                                        scalar2=float(fnp(-2.0 / 3.0)),
                                        op0=ALU.mult, op1=ALU.add)
            emit(c_sv)

            def c_s0(v, hs):
                nc.vector.tensor_tensor(out=v["s"], in0=v["sv"][:],
                                        in1=v["u"][:], op=ALU.add)
            emit(c_s0)

            def c_smax(v, hs):
                nc.vector.tensor_scalar_max(v["s"], v["s"], 0.0)
            emit(c_smax)

            for _ in range(NEWTON_ITERS):
                def n_one(v, hs):
                    v["one"] = tmp()
                    nc.vector.tensor_scalar_add(v["one"][:], v["s"], 1.0)
                emit(n_one)

                def n_o2(v, hs):
                    v["o2"] = tmp()
                    nc.vector.tensor_tensor(out=v["o2"][:], in0=v["one"][:],
                                            in1=v["one"][:], op=ALU.mult)
                emit(n_o2)

                def n_num(v, hs):
                    v["num"] = tmp()
                    nc.vector.tensor_tensor(out=v["num"][:], in0=v["s"],
                                            in1=v["o2"][:], op=ALU.mult)
                emit(n_num)

                def n_num2(v, hs):
                    nc.vector.tensor_tensor(out=v["num"][:], in0=v["num"][:],
                                            in1=v["r"], op=ALU.subtract)
                emit(n_num2)

                def n_t3(v, hs):
                    v["t3"] = tmp()
                    nc.vector.tensor_scalar(out=v["t3"][:], in0=v["s"],
                                            scalar1=3.0, scalar2=1.0,
                                            op0=ALU.mult, op1=ALU.add)
                emit(n_t3)

                def n_den(v, hs):
                    v["den"] = tmp()
                    nc.vector.tensor_tensor(out=v["den"][:], in0=v["one"][:],
                                            in1=v["t3"][:], op=ALU.mult)
                emit(n_den)

                def n_rden(v, hs):
                    v["rden"] = tmp()
                    nc.vector.reciprocal(out=v["rden"][:], in_=v["den"][:])
                emit(n_rden)

                def n_quo(v, hs):
                    v["quo"] = tmp()
                    nc.vector.tensor_tensor(out=v["quo"][:], in0=v["num"][:],
                                            in1=v["rden"][:], op=ALU.mult)
                emit(n_quo)

                def n_s(v, hs):
                    nc.vector.tensor_tensor(out=v["s"], in0=v["s"],
                                            in1=v["quo"][:], op=ALU.subtract)
                emit(n_s)

                def n_smax(v, hs):
                    nc.vector.tensor_scalar_max(v["s"], v["s"], 0.0)
                emit(n_smax)

            def f_onep(v, hs):
                v["onep"] = tmp()
                nc.vector.tensor_scalar_add(v["onep"][:], v["s"], 1.0)
            emit(f_onep)

            def f_cf(v, hs):
                v["cf"] = tmp()
                nc.vector.reciprocal(out=v["cf"][:], in_=v["onep"][:])
            emit(f_cf)

            def f_chi(v, hs):
                nc.vector.tensor_copy(out=cstack[0:GBLK, hs], in_=v["cf"][:])
            emit(f_chi)

            def f_clo(v, hs):
                nc.vector.tensor_tensor(out=cstack[32:32 + GBLK, hs],
                                        in0=v["cf"][:],
                                        in1=cstack[0:GBLK, hs],
                                        op=ALU.subtract)
            emit(f_clo)
            return cstack

        # ---- software-pipelined emission ----
        # Per-block stage chain: L1(PE) -> tanh1(ACT) -> L2(PE) -> tanh2(ACT)
        # -> L3(PE) -> q+bias(ACT) -> qsq(GPSIMD) -> r(PE). Emitting stage s
        # of block T-s at tick T gives every cross-engine edge a one-tick
        # slack, so the in-order engine queues stream without head-of-line
        # stalls. Phase 3 of group g-1 is interleaved into group g's ticks
        # (offset OFF3) so its cb-matmuls sit in the PE queue only after
        # phase 2 of g-1 has had time to finish on DVE/ACT.
        STAGES = 8
        OFF3 = 9

        st_h1p, st_h1, st_h2p, st_h2, st_qp, st_qsq = {}, {}, {}, {}, {}, {}
        st_xa, st_xb = {}, {}
        st_cb, st_ut = {}, {}

        def ph1_tick(g, T, rps):
            # stage 0: loads + L1
            j = T
            if 0 <= j < GBLK:
                b = g * GBLK + j
                su, so = divmod(b, SUPER)
                if so == 0:
                    xa = xpool.tile([128, SUPN], mmdt, tag="xa", name="xa")
                    nc.sync.dma_start(out=xa[:], in_=zTt_d[su, 0:128, :])
                    xb = xpool.tile([65, SUPN], mmdt, tag="xb", name="xb")
                    nc.sync.dma_start(out=xb[:], in_=zTt_d[su, 128:193, :])
                    st_xa[su], st_xb[su] = xa, xb
                ns = slice(so * BLKN, (so + 1) * BLKN)
                h1p = mmps.tile([HID, BLKN], f32, tag="h1p", bufs=2, name="h1p")
                nc.tensor.matmul(h1p[:], w1a[:], st_xa[su][:, ns],
                                 start=True, stop=False)
                nc.tensor.matmul(h1p[:], w1b[:], st_xb[su][:, ns],
                                 start=False, stop=True)
                st_h1p[j] = h1p
            # stage 1: tanh1
            j = T - 1
            if 0 <= j < GBLK:
                h1 = hpool.tile([HID, BLKN], mmdt, tag="h1", bufs=3, name="h1")
                nc.scalar.activation(out=h1[:], in_=st_h1p.pop(j)[:],
                                     func=FT.Tanh, bias=b1[:])
                st_h1[j] = h1
            # stage 2: L2
            j = T - 2
            if 0 <= j < GBLK:
                h2p = mmps.tile([HID, BLKN], f32, tag="h2p", bufs=2, name="h2p")
                nc.tensor.matmul(h2p[:], w2[:], st_h1.pop(j)[:],
                                 start=True, stop=True)
                st_h2p[j] = h2p
            # stage 3: tanh2
            j = T - 3
            if 0 <= j < GBLK:
                h2 = hpool.tile([HID, BLKN], mmdt, tag="h2", bufs=3, name="h2")
                nc.scalar.activation(out=h2[:], in_=st_h2p.pop(j)[:],
                                     func=FT.Tanh, bias=b2[:])
                st_h2[j] = h2
            # stage 4: L3
            j = T - 4
            if 0 <= j < GBLK:
                qp = mmps.tile([CD, BLKN], f32, tag="qp", bufs=1, name="qp")
                nc.tensor.matmul(qp[:], w3[:], st_h2.pop(j)[:],
                                 start=True, stop=True)
                st_qp[j] = qp
            # stage 5: q = q- + b3 (PSUM -> persistent SBUF), on DVE
            j = T - 5
            if 0 <= j < GBLK:
                n0 = (g * GBLK + j) * BLKN
                nc.vector.tensor_scalar(out=qsb[:, n0:n0 + BLKN],
                                        in0=st_qp.pop(j)[:], scalar1=b3c[:],
                                        scalar2=None, op0=ALU.add)
            # stage 6: qsq on gpsimd
            j = T - 6
            if 0 <= j < GBLK:
                n0 = (g * GBLK + j) * BLKN
                qsq = hpool.tile([CD, BLKN], mmdt, tag="qsq", bufs=3,
                                 name="qsq")
                nc.gpsimd.tensor_tensor(out=qsq[:], in0=qsb[:, n0:n0 + BLKN],
                                        in1=qsb[:, n0:n0 + BLKN], op=ALU.mult)
                st_qsq[j] = qsq
            # stage 7: r accumulation
            j = T - 7
            if 0 <= j < GBLK:
                nc.tensor.matmul(rps[:], rsel[:, GBLK * j:GBLK * (j + 1)],
                                 st_qsq.pop(j)[:], start=(j == 0),
                                 stop=(j == GBLK - 1))

        def ph3_tick(g, T, cstack):
            # stage A: cb broadcast matmul
            j = T
            if 0 <= j < GBLK:
                b = g * GBLK + j
                su, so = divmod(b, SUPER)
                if so == 0:
                    st_ut[su] = opool.tile([CD, SUPN], f32, tag="ut", bufs=3,
                                           name="ut")
                cb = cps_pool.tile([CD, BLKN], f32, tag="cb", name="cb")
                nc.tensor.matmul(cb[:], sels[:, CD * j:CD * (j + 1)],
                                 cstack[:], start=True, stop=True)
                st_cb[j] = cb
            # stage B: ut multiply + store
            j = T - 1
            if 0 <= j < GBLK:
                b = g * GBLK + j
                n0 = b * BLKN
                su, so = divmod(b, SUPER)
                ns = slice(so * BLKN, (so + 1) * BLKN)
                nc.vector.tensor_tensor(out=st_ut[su][:, ns],
                                        in0=qsb[:, n0:n0 + BLKN],
                                        in1=st_cb.pop(j)[:], op=ALU.mult)
                if so == SUPER - 1 or j == GBLK - 1:
                    nc.sync.dma_start(out=uT_d[su], in_=st_ut.pop(su)[:])

        for _rep in range(REPEAT):
            prev = None                      # (g-1, cstack) awaiting phase 3
            for g in range(NG):
                rps = rps_pool.tile([GBLK, BLKN], f32, tag="rps", name="rps")
                for T in range(GBLK + STAGES):
                    ph1_tick(g, T, rps)
                    if prev is not None:
                        ph3_tick(prev[0], T - OFF3, prev[1])
                cstack = phase2(rps)
                if prev is not None:
                    for T in range(GBLK + STAGES - OFF3, GBLK + 1):
                        ph3_tick(prev[0], T, prev[1])
                prev = (g, cstack)
            # drain last group's phase 3
            for T in range(GBLK + 1):
                ph3_tick(prev[0], T, prev[1])

    nc.compile()
    return nc


def _host_constants(W1, b1, W2, b2, W3, b3):
    S = _scatter_matrix()
    W1z = W1[1:, :].astype(np.float32)          # [192, 100]
    W1t = W1[0:1, :].astype(np.float32)         # [1, 100]
    w1a = np.ascontiguousarray(W1z[0:128])
    w1b = np.ascontiguousarray(np.concatenate([W1z[128:192], W1t], axis=0))
    w3 = np.ascontiguousarray(-(W3.astype(np.float64) @ S)).astype(np.float32)
    b3n = (-(b3.astype(np.float64) @ S)).astype(np.float32).reshape(CD, 1)
    rsel = np.zeros((CD, GBLK * GBLK), np.float32)
    for j in range(GBLK):
        rsel[:, GBLK * j + j] = 1.0
    sels = np.zeros((32 + GBLK, GBLK * CD), np.float32)
    for j in range(GBLK):
        sels[j, CD * j:CD * (j + 1)] = 1.0            # hi part
        sels[32 + j, CD * j:CD * (j + 1)] = 1.0       # lo part (aligned @32)
    return {
        "w1a": w1a,
        "w1b": w1b,
        "w2": np.ascontiguousarray(W2.astype(np.float32)),
        "w3": w3,
        "b1": np.ascontiguousarray(b1.astype(np.float32).reshape(HID, 1)),
        "b2": np.ascontiguousarray(b2.astype(np.float32).reshape(HID, 1)),
        "b3": np.ascontiguousarray(b3n),
        "rsel": rsel,
        "sels": sels,
    }


def _shard_inputs(z, t, consts):
    in_maps = []
    for c in range(N_CORES):
        sl = slice(c * BC, (c + 1) * BC)
        m = dict(consts)
        xt = np.concatenate([z[sl].T, t[sl].reshape(1, BC)], axis=0)  # [193,BC]
        xt = xt.reshape(SD + 1, NSUP, SUPN).transpose(1, 0, 2)
        m["zTt"] = np.ascontiguousarray(xt)       # [NSUP, 193, SUPN]
        in_maps.append(m)
    return in_maps


def _unshard_output(uT):
    """uT [NSUP, CD, SUPN] -> [BC, CD]."""
    full = np.asarray(uT).transpose(1, 0, 2).reshape(CD, BC)
    return np.ascontiguousarray(full.T)


def _get_program():
    key = (USE_F32R, NEWTON_ITERS, NG, REPEAT, SUPER)
    if key not in _PROG_CACHE:
        _PROG_CACHE[key] = _build_program()
    return _PROG_CACHE[key]


def kernel(z, t, W1, b1, W2, b2, W3, b3, _trace=False):
    from concourse.bass_utils import run_bass_kernel_spmd

    z = np.asarray(z, np.float32)
    t = np.asarray(t, np.float32)
    consts = _host_constants(np.asarray(W1), np.asarray(b1), np.asarray(W2),
                             np.asarray(b2), np.asarray(W3), np.asarray(b3))
    nc = _get_program()
    in_maps = _shard_inputs(z, t, consts)
    res = run_bass_kernel_spmd(nc, in_maps, list(range(N_CORES)), trace=_trace)
    outs = [_unshard_output(res.results[c]["uT"]) for c in range(N_CORES)]
    u = np.concatenate(outs, axis=0).astype(np.float32)
    if _trace:
        return u, res
    return u


def _make_runner(in_maps):
    """Build the sharded PJRT callable (same lowering as run_bass_via_pjrt)
    and put inputs on device once, for steady-state timing."""
    import jax
    import numpy as _np
    from jax.sharding import Mesh, PartitionSpec
    from jax.experimental.shard_map import shard_map
    import concourse.mybir as mybir
    from concourse import bass2jax

    nc = _get_program()
    bass2jax.install_neuronx_cc_hook()

    partition_name = (nc.partition_id_tensor.name
                      if nc.partition_id_tensor else None)
    in_names, out_names, out_avals, zero_outs = [], [], [], []
    for alloc in nc.m.functions[0].allocations:
        if not isinstance(alloc, mybir.MemoryLocationSet):
            continue
        name = alloc.memorylocations[0].name
        if alloc.kind == "ExternalInput":
            if name != partition_name:
                in_names.append(name)
        elif alloc.kind == "ExternalOutput":
            shape = list(alloc.tensor_shape)
            dt = mybir.dt.np(alloc.dtype)
            out_names.append(name)
            out_avals.append(jax.core.ShapedArray(shape, dt))
            zero_outs.append(_np.zeros(shape, dt))
    in_names_full = in_names + out_names
    if partition_name is not None:
        in_names_full.append(partition_name)

    def _body(*args):
        operands = list(args)
        if partition_name is not None:
            operands.append(bass2jax.partition_id_tensor())
        outs = bass2jax._bass_exec_p.bind(
            *operands,
            out_avals=tuple(out_avals),
            in_names=tuple(in_names_full),
            out_names=tuple(out_names),
            lowering_input_output_aliases=(),
            sim_require_finite=True,
            sim_require_nnan=True,
            nc=nc,
        )
        return tuple(outs)

    devices = jax.devices()[:N_CORES]
    mesh = Mesh(np.asarray(devices), ("core",))
    nin = len(in_names) + len(zero_outs)
    fn = jax.jit(shard_map(_body, mesh=mesh,
                           in_specs=(PartitionSpec("core"),) * nin,
                           out_specs=(PartitionSpec("core"),) * len(out_names),
                           check_rep=False), keep_unused=True)
    concat = [_np.concatenate([in_maps[c][n] for c in range(N_CORES)], axis=0)
              for n in in_names]
    concat += [_np.zeros((N_CORES * z.shape[0], *z.shape[1:]), z.dtype)
               for z in zero_outs]
    sh = jax.sharding.NamedSharding(mesh, PartitionSpec("core"))
    dev_in = [jax.device_put(a, sh) for a in concat]
    return fn, dev_in, out_names


def bench(z, t, W1, b1, W2, b2, W3, b3, iters=20):
    """Returns (per-iteration wall ns, outputs dict) at steady state."""
    import time as _time
    import jax
    z = np.asarray(z, np.float32)
    t = np.asarray(t, np.float32)
    consts = _host_constants(np.asarray(W1), np.asarray(b1), np.asarray(W2),
                             np.asarray(b2), np.asarray(W3), np.asarray(b3))
    in_maps = _shard_inputs(z, t, consts)
    fn, dev_in, out_names = _make_runner(in_maps)
    out = fn(*dev_in)
    jax.block_until_ready(out)
    t0 = _time.perf_counter()
    for _ in range(iters):
        out = fn(*dev_in)
    jax.block_until_ready(out)
    t1 = _time.perf_counter()
    return (t1 - t0) / iters * 1e9, dict(zip(out_names, out))
